# revision 26
# baseline (speedup 1.0000x reference)
"""GCN message-passing (gather + segment_sum + linear + PReLU) on 8 Trainium2 cores.

Strategy: shard destination nodes across cores.  Since segment_sum commutes
with the linear layer, raw seq features are aggregated first and W applied
after aggregation:

    out = prelu(segsum(val * seq[src]) @ W.T + bias)

Destination nodes are relabeled into (8 cores) x (wpc windows) x (128 slots),
degree-balanced so every window has a bounded number of incident edges.
Edges are grouped into chunks of 128 (one edge per SBUF partition).  Source
rows are fetched with dma_gather (int16 indices -> the seq table is split
into <=32767-row buckets; a window's edges are grouped by bucket).  For each
chunk a selection matrix S^T[e, s] = val[e] * (slot[e] == s) is built on the
DVE and the PE accumulates G.T @ S^T into PSUM [feat, slot] over the window.
One more matmul with W^T gives [slot, out_ft]; PReLU on the way out.
"""

import os
import sys

import numpy as np

for _p in ("/opt/trn_rl_repo", "/root/.axon_site/_ro/trn_rl_repo"):
    if os.path.isdir(_p) and _p not in sys.path:
        sys.path.insert(0, _p)

from concourse import bacc, bass, mybir, tile  # noqa: E402
from concourse.bass_utils import run_bass_kernel_spmd  # noqa: E402

P = 128
N_CORES = 8
MAX_BUCKET = 25000  # rows per gather table (int16 index limit is 32767)
_BF16_NP = mybir.dt.np(mybir.dt.bfloat16)

_prog_cache: dict = {}
LAST_RESULTS = None  # BassKernelResults of the most recent kernel() call


def _build_program(bpad: int, nb: int, cbs: tuple, wpc: int, wpg: int,
                   alpha: float, has_bias: bool, reps: int = 1) -> "bacc.Bacc":
    """cbs: per-bucket chunk counts (C_b).  Chunk grid per group of wpg
    windows: [b0: w0 chunks.. w_{wpg-1} chunks][b1: ...] ...  One dma_gather
    call per (group, bucket) covers its contiguous wpg*C_b chunk run."""
    dt = mybir.dt
    C = sum(cbs)
    nch = wpc * C
    offs = np.concatenate([[0], np.cumsum(cbs)]).astype(int)
    n_groups = wpc // wpg

    nq = int(os.environ.get("GCN_NQUEUES", "4"))
    nc = bacc.Bacc(num_swdge_queues=nq)
    tabs = [nc.declare_dram_parameter(f"tab{b}", [bpad, P], dt.bfloat16,
                                      isOutput=False) for b in range(nb)]
    idxs_d = nc.declare_dram_parameter("idxs", [P, nch * 8], dt.int16, isOutput=False)
    counts_d = nc.declare_dram_parameter("counts", [1, wpc * nb], dt.int32, isOutput=False)
    slots_d = nc.declare_dram_parameter("slots", [P, nch], dt.bfloat16, isOutput=False)
    vals_d = nc.declare_dram_parameter("vals", [P, nch], dt.bfloat16, isOutput=False)
    iota_d = nc.declare_dram_parameter("iota", [P, P], dt.bfloat16, isOutput=False)
    wt_d = nc.declare_dram_parameter("wt", [P, P], dt.bfloat16, isOutput=False)
    if has_bias:
        bias_d = nc.declare_dram_parameter("biasb", [P, P], dt.float32, isOutput=False)
    out_d = nc.declare_dram_parameter("out", [wpc * P, P], dt.float32, isOutput=True)

    with tile.TileContext(nc) as tc:
        with (
            tc.tile_pool(name="const", bufs=1) as constp,
            tc.tile_pool(name="edges", bufs=1) as edgep,
            tc.tile_pool(name="gat", bufs=3) as gatp,
            tc.tile_pool(name="cmp", bufs=2) as cmpp,
            tc.tile_pool(name="smat", bufs=3) as smatp,
            tc.tile_pool(name="o1", bufs=3) as o1p,
            tc.tile_pool(name="of", bufs=4) as ofp,
            tc.tile_pool(name="ps1", bufs=3, space="PSUM") as ps1p,
            tc.tile_pool(name="ps2", bufs=2, space="PSUM") as ps2p,
        ):
            iota_sb = constp.tile([P, P], dt.bfloat16, tag="iota")
            nc.sync.dma_start(out=iota_sb[:], in_=iota_d[:])
            wt_sb = constp.tile([P, P], dt.bfloat16, tag="wt")
            nc.sync.dma_start(out=wt_sb[:], in_=wt_d[:])
            if has_bias:
                bias_sb = constp.tile([P, P], dt.float32, tag="bias")
                nc.sync.dma_start(out=bias_sb[:], in_=bias_d[:])
            idxs_sb = edgep.tile([P, nch * 8], dt.int16, tag="idxs")
            nc.sync.dma_start(out=idxs_sb[:], in_=idxs_d[:])
            counts_sb = edgep.tile([1, wpc * nb], dt.int32, tag="counts")
            nc.sync.dma_start(out=counts_sb[:], in_=counts_d[:])
            regs = [nc.alloc_register(mybir.EngineType.Pool, name=f"gcnt{i}")
                    for i in range(8)]
            slots_sb = edgep.tile([P, nch], dt.bfloat16, tag="slots")
            nc.sync.dma_start(out=slots_sb[:], in_=slots_d[:])
            vals_sb = edgep.tile([P, nch], dt.bfloat16, tag="vals")
            nc.sync.dma_start(out=vals_sb[:], in_=vals_d[:])

            gc = wpg * C  # chunks per group
            # zero-fill gather slots once: skipped (-1) rows leave stale SBUF
            # which S multiplies by 0 — must be finite, not virgin-NaN bits
            for _i in range(3):
                gz = gatp.tile([P, gc * P], dt.bfloat16, tag="g")
                nc.vector.memset(gz[:], 0)
            for _rep in range(reps):
              for g in range(n_groups):
                gt = gatp.tile([P, gc * P], dt.bfloat16, tag="g")
                for wi in range(wpg):
                    w = g * wpg + wi
                    for b in range(nb):
                        cb = cbs[b]
                        if cb == 0:
                            continue
                        i_call = w * nb + b
                        base = wpg * offs[b] + wi * cb  # chunk offset in group
                        ni = cb * P
                        reg = regs[i_call % 8]
                        nc.gpsimd.reg_load(reg, counts_sb[0:1, i_call:i_call + 1])
                        nc.gpsimd.dma_gather(
                            out_ap=gt[:, base * P:(base + cb) * P].rearrange(
                                "p (k s) -> p k s", s=P),
                            in_ap=tabs[b][:],
                            idxs_ap=idxs_sb[:, (g * gc + base) * 8:(g * gc + base) * 8 + ni // 16],
                            num_idxs=ni,
                            num_idxs_reg=reg,
                            elem_size=P,
                            single_packet=(ni <= 1024),
                            queue_num=i_call % nq,
                        )
                if os.environ.get("GCN_STAGE") == "gather":
                    for wi in range(wpg):
                        w = g * wpg + wi
                        of0 = ofp.tile([P, P], dt.float32, tag="of")
                        nc.vector.tensor_copy(out=of0[:], in_=gt[:, wi * P:(wi + 1) * P])
                        nc.sync.dma_start(out=out_d[w * P:(w + 1) * P, :], in_=of0[:])
                    continue
                c0 = g * gc
                cmp_t = cmpp.tile([P, gc * P], dt.bfloat16, tag="cmp")
                s_t = smatp.tile([P, gc * P], dt.bfloat16, tag="s")
                nc.vector.tensor_tensor(
                    out=cmp_t[:].rearrange("p (g s) -> p g s", s=P),
                    in0=slots_sb[:, c0:c0 + gc, None].to_broadcast([P, gc, P]),
                    in1=iota_sb[:, None, :].to_broadcast([P, gc, P]),
                    op=mybir.AluOpType.is_equal,
                )
                nc.vector.tensor_tensor(
                    out=s_t[:].rearrange("p (g s) -> p g s", s=P),
                    in0=cmp_t[:].rearrange("p (g s) -> p g s", s=P),
                    in1=vals_sb[:, c0:c0 + gc, None].to_broadcast([P, gc, P]),
                    op=mybir.AluOpType.mult,
                )
                for wi in range(wpg):
                    w = g * wpg + wi
                    # this window's chunk columns within the group tile
                    cols = []
                    for b in range(nb):
                        cb = cbs[b]
                        base = wpg * offs[b] + wi * cb
                        cols.extend(range(base, base + cb))
                    ps1 = ps1p.tile([P, P], dt.float32, tag="ps1")
                    for ci, c in enumerate(cols):
                        off = c * P
                        nc.tensor.matmul(
                            out=ps1[:],
                            lhsT=gt[:, off:off + P],
                            rhs=s_t[:, off:off + P],
                            start=(ci == 0),
                            stop=(ci == len(cols) - 1),
                        )
                    o1 = o1p.tile([P, P], dt.bfloat16, tag="o1")
                    nc.scalar.copy(out=o1[:], in_=ps1[:])
                    ps2 = ps2p.tile([P, P], dt.float32, tag="ps2")
                    nc.tensor.matmul(out=ps2[:], lhsT=o1[:], rhs=wt_sb[:],
                                     start=True, stop=True)
                    if has_bias:
                        tb = ofp.tile([P, P], dt.float32, tag="tb")
                        nc.vector.tensor_tensor(out=tb[:], in0=ps2[:], in1=bias_sb[:],
                                                op=mybir.AluOpType.add)
                        pre = tb
                    else:
                        pre = ps2
                    tm = ofp.tile([P, P], dt.float32, tag="tm")
                    nc.vector.tensor_scalar_mul(tm[:], pre[:], float(alpha))
                    of = ofp.tile([P, P], dt.float32, tag="of")
                    nc.vector.tensor_tensor(out=of[:], in0=pre[:], in1=tm[:],
                                            op=mybir.AluOpType.max)
                    nc.sync.dma_start(out=out_d[w * P:(w + 1) * P, :], in_=of[:])
    nc.compile()
    return nc


def _prep(seq2d, edge_val, edge_src, edge_dst):
    """Host-side: balance nodes into windows, bucket edges into chunk slots.

    Returns (per_core_arrays, node_row, wpc, nb, bpad, cbs)."""
    n = seq2d.shape[0]
    wpc = -(-n // (P * N_CORES))       # windows per core
    wpc = -(-wpc // 4) * 4             # round up so wpg=4 divides it
    tw = wpc * N_CORES                 # total windows
    nb = -(-n // MAX_BUCKET)           # gather-table buckets
    bpad = -(-n // nb)

    dst = edge_dst.astype(np.int64)
    src = edge_src.astype(np.int64)
    deg = np.bincount(dst, minlength=n)

    # serpentine assignment of degree-sorted nodes -> (window, slot)
    order = np.argsort(-deg, kind="stable")
    idx = np.arange(n)
    rnd = idx // tw
    pos = idx % tw
    wins = np.where(rnd % 2 == 0, pos, tw - 1 - pos)
    node_w = np.empty(n, np.int64)
    node_s = np.empty(n, np.int64)
    node_w[order] = wins
    node_s[order] = rnd
    node_row = node_w * P + node_s  # row in concatenated all-core output

    ew = node_w[dst]                # window of each edge
    ecore = ew // wpc
    ewl = ew % wpc
    eslot = node_s[dst]
    ebuck = src // bpad
    eloc = src % bpad

    wpg = 4 if wpc % 4 == 0 else (2 if wpc % 2 == 0 else 1)

    # per-(core, window, bucket) counts -> global per-bucket chunk maxima
    seg_of_edge = (ecore * wpc + ewl) * nb + ebuck
    cnt = np.bincount(seg_of_edge, minlength=tw * nb).reshape(tw, nb)
    cbs = tuple(int(x) for x in np.maximum(-(-cnt.max(axis=0) // P), 0))
    C = sum(cbs)
    offs = np.concatenate([[0], np.cumsum(cbs)]).astype(int)
    nch = wpc * C

    per_core = []
    for c in range(N_CORES):
        m = ecore == c
        key = ewl[m] * nb + ebuck[m]
        o = np.argsort(key, kind="stable")
        key = key[o]
        wl = ewl[m][o]
        bk = ebuck[m][o]
        sl = eslot[m][o]
        lo = eloc[m][o]
        vl = edge_val[m][o]
        scnt = np.bincount(key, minlength=wpc * nb)
        sstart = np.concatenate([[0], np.cumsum(scnt)[:-1]])
        pos_in_seg = np.arange(len(wl)) - sstart[key]
        g = wl // wpg
        wi = wl % wpg
        chunk = g * (wpg * C) + wpg * offs[bk] + wi * np.asarray(cbs)[bk] \
            + pos_in_seg // P
        lane = pos_in_seg % P
        assert (pos_in_seg < np.asarray(cbs)[bk] * P).all()

        flat = chunk * P + lane
        slot_a = np.zeros(nch * P, np.float32)
        val_a = np.zeros(nch * P, np.float32)
        slot_a[flat] = sl
        val_a[flat] = vl

        # idx array: call (w, b) is a contiguous run of C_b chunks; within the
        # call, element i sits at [i % 16, call_col0 + i // 16].  Unfilled
        # tail entries stay -1 (dma_gather skips a -1 suffix: no transfer).
        run_start_chunk = g * (wpg * C) + wpg * offs[bk] + wi * np.asarray(cbs)[bk]
        i_in_call = pos_in_seg
        col = run_start_chunk * 8 + i_in_call // 16
        row = i_in_call % 16
        idx_a = np.full((16, nch * 8), -1, np.int16)
        idx_a[row, col] = lo
        counts = np.maximum(scnt, 1).astype(np.int32)
        # empty (w, b) calls: count forced to 1 -> make their first idx valid
        for seg in np.nonzero(scnt == 0)[0]:
            wl0, b0 = seg // nb, seg % nb
            if cbs[b0] == 0:
                counts[seg] = 0
                continue
            rs = (wl0 // wpg) * (wpg * C) + wpg * offs[b0] + (wl0 % wpg) * cbs[b0]
            idx_a[0, rs * 8] = 0
        idx_full = np.tile(idx_a, (8, 1))

        per_core.append((
            idx_full,
            np.ascontiguousarray(slot_a.reshape(nch, P).T).astype(_BF16_NP),
            np.ascontiguousarray(val_a.reshape(nch, P).T).astype(_BF16_NP),
            counts.reshape(1, wpc * nb),
        ))
    return per_core, node_row, wpc, nb, bpad, cbs


def kernel(seq, W, bias, prelu_a, edge_val, edge_src, edge_dst):
    global LAST_RESULTS
    seq = np.asarray(seq)
    W = np.asarray(W, dtype=np.float32)
    bias = np.asarray(bias, dtype=np.float32)
    alpha = float(np.asarray(prelu_a).reshape(-1)[0])
    assert 0.0 <= alpha <= 1.0, "prelu slope must be in [0,1] for the max() trick"
    edge_val = np.asarray(edge_val, dtype=np.float32)

    seq2d = np.ascontiguousarray(seq.reshape(-1, P).astype(np.float32))
    n = seq2d.shape[0]

    per_core, node_row, wpc, nb, bpad, cbs = _prep(
        seq2d, edge_val, np.asarray(edge_src), np.asarray(edge_dst))
    has_bias = bool(np.any(bias != 0.0))
    wpg = 4 if wpc % 4 == 0 else (2 if wpc % 2 == 0 else 1)

    cfg = (bpad, nb, cbs, wpc, wpg, alpha, has_bias)
    if cfg not in _prog_cache:
        _prog_cache[cfg] = _build_program(*cfg)
    nc = _prog_cache[cfg]

    seq_tab = seq2d.astype(_BF16_NP)
    tabs = {}
    for b in range(nb):
        t = seq_tab[b * bpad:(b + 1) * bpad]
        if t.shape[0] < bpad:
            t = np.vstack([t, np.zeros((bpad - t.shape[0], P), _BF16_NP)])
        tabs[f"tab{b}"] = np.ascontiguousarray(t)
    iota = np.tile(np.arange(P, dtype=np.float32), (P, 1)).astype(_BF16_NP)
    wt = np.ascontiguousarray(W.T).astype(_BF16_NP)
    in_maps = []
    for c in range(N_CORES):
        idx_a, slot_a, val_a, counts = per_core[c]
        m = dict(tabs)
        m.update({
            "idxs": idx_a,
            "slots": slot_a,
            "vals": val_a,
            "counts": counts,
            "iota": iota,
            "wt": wt,
        })
        if has_bias:
            m["biasb"] = np.ascontiguousarray(
                np.tile(bias.astype(np.float32), (P, 1)))
        in_maps.append(m)

    res = run_bass_kernel_spmd(nc, in_maps, list(range(N_CORES)))
    LAST_RESULTS = res

    flat = np.concatenate([res.results[c]["out"] for c in range(N_CORES)], axis=0)
    out = flat[node_row].astype(np.float32)
    _LAST_RUN["nc"] = nc
    _LAST_RUN["in_maps"] = in_maps
    _LAST_RUN["cfg"] = cfg
    return out.reshape(seq.shape[0], n, P) if seq.ndim == 3 else out


_LAST_RUN: dict = {}


def _time_program(nc, in_maps, iters: int = 50) -> dict:
    """Execute a program with device-resident inputs; return per-call wall
    times (ns).  'pipelined' issues all calls async then blocks once."""
    import time

    import jax
    from jax.sharding import Mesh, PartitionSpec
    from jax.experimental.shard_map import shard_map
    from concourse import bass2jax, mybir as mb

    bass2jax.install_neuronx_cc_hook()

    partition_name = nc.partition_id_tensor.name if nc.partition_id_tensor else None
    in_names, out_names, out_avals, zero_outs = [], [], [], []
    for alloc in nc.m.functions[0].allocations:
        if not isinstance(alloc, mb.MemoryLocationSet):
            continue
        name = alloc.memorylocations[0].name
        if alloc.kind == "ExternalInput":
            if name != partition_name:
                in_names.append(name)
        elif alloc.kind == "ExternalOutput":
            out_names.append(name)
            shape = tuple(alloc.tensor_shape)
            dtype = mb.dt.np(alloc.dtype)
            out_avals.append(jax.core.ShapedArray(shape, dtype))
            zero_outs.append(np.zeros(shape, dtype))
    n_params = len(in_names)
    all_in = list(in_names) + list(out_names)

    def _body(*args):
        operands = list(args)
        if partition_name is not None:
            operands.append(bass2jax.partition_id_tensor())
        return tuple(bass2jax._bass_exec_p.bind(
            *operands,
            out_avals=tuple(out_avals),
            in_names=tuple(all_in + ([partition_name] if partition_name else [])),
            out_names=tuple(out_names),
            lowering_input_output_aliases=(),
            sim_require_finite=True,
            sim_require_nnan=True,
            nc=nc,
        ))

    devices = jax.devices()[:N_CORES]
    mesh = Mesh(np.asarray(devices), ("core",))
    nin = n_params + len(zero_outs)
    sharded = jax.jit(shard_map(
        _body, mesh=mesh,
        in_specs=(PartitionSpec("core"),) * nin,
        out_specs=(PartitionSpec("core"),) * len(out_names),
        check_rep=False), keep_unused=True)

    sh = jax.sharding.NamedSharding(mesh, PartitionSpec("core"))
    dev_in = [jax.device_put(
        np.concatenate([np.asarray(in_maps[c][nm]) for c in range(N_CORES)], axis=0), sh)
        for nm in in_names]
    dev_zero = [jax.device_put(
        np.zeros((N_CORES * z.shape[0], *z.shape[1:]), z.dtype), sh)
        for z in zero_outs]

    out = sharded(*dev_in, *dev_zero)
    jax.block_until_ready(out)

    t0 = time.perf_counter()
    outs = [sharded(*dev_in, *dev_zero) for _ in range(iters)]
    jax.block_until_ready(outs)
    t_pipe = (time.perf_counter() - t0) / iters

    return {"pipelined_ns": t_pipe * 1e9}


def bench(iters: int = 50, reps: int = 3) -> dict:
    """Slope-based HW timing of the last kernel() call: time the program and a
    variant with the whole pipeline repeated `reps` times inside one NEFF;
    (T_reps - T_1) / (reps - 1) cancels per-execute dispatch overhead."""
    nc = _LAST_RUN["nc"]
    in_maps = _LAST_RUN["in_maps"]
    cfg = _LAST_RUN["cfg"]
    t1 = _time_program(nc, in_maps, iters)["pipelined_ns"]
    key = cfg + (reps,)
    if key not in _prog_cache:
        _prog_cache[key] = _build_program(*cfg, reps=reps)
    ncr = _prog_cache[key]
    tr = _time_program(ncr, in_maps, iters)["pipelined_ns"]
    slope = (tr - t1) / (reps - 1)
    return {"pipelined_ns": t1, "reps_ns": tr, "slope_ns": slope}


# revision 28
# speedup vs baseline: 1.4943x; 1.4943x over previous
"""GCN message-passing (gather + segment_sum + linear + PReLU) on 8 Trainium2 cores.

Strategy: shard destination nodes across cores.  Since segment_sum commutes
with the linear layer, raw seq features are aggregated first and W applied
after aggregation:

    out = prelu(segsum(val * seq[src]) @ W.T + bias)

Destination nodes are relabeled into (8 cores) x (wpc windows) x (128 slots),
degree-balanced so every window has a bounded number of incident edges.
Edges are grouped into chunks of 128 (one edge per SBUF partition).  Source
rows are fetched with dma_gather (int16 indices -> the seq table is split
into <=32767-row buckets; a window's edges are grouped by bucket).  For each
chunk a selection matrix S^T[e, s] = val[e] * (slot[e] == s) is built on the
DVE and the PE accumulates G.T @ S^T into PSUM [feat, slot] over the window.
One more matmul with W^T gives [slot, out_ft]; PReLU on the way out.
"""

import os
import sys

import numpy as np

for _p in ("/opt/trn_rl_repo", "/root/.axon_site/_ro/trn_rl_repo"):
    if os.path.isdir(_p) and _p not in sys.path:
        sys.path.insert(0, _p)

from concourse import bacc, bass, mybir, tile  # noqa: E402
from concourse.bass_utils import run_bass_kernel_spmd  # noqa: E402

P = 128
N_CORES = 8
MAX_BUCKET = 25000  # rows per gather table (int16 index limit is 32767)
_BF16_NP = mybir.dt.np(mybir.dt.bfloat16)

_prog_cache: dict = {}
LAST_RESULTS = None  # BassKernelResults of the most recent kernel() call


def _build_program(bpad: int, nb: int, cbs: tuple, wpc: int, wpg: int,
                   alpha: float, has_bias: bool, reps: int = 1) -> "bacc.Bacc":
    """cbs: per-bucket chunk counts (C_b).  Chunk grid per group of wpg
    windows: [b0: w0 chunks.. w_{wpg-1} chunks][b1: ...] ...  One dma_gather
    call per (group, bucket) covers its contiguous wpg*C_b chunk run."""
    dt = mybir.dt
    C = sum(cbs)
    nch = wpc * C
    offs = np.concatenate([[0], np.cumsum(cbs)]).astype(int)
    n_groups = wpc // wpg

    nq = int(os.environ.get("GCN_NQUEUES", "4"))
    nc = bacc.Bacc(num_swdge_queues=nq)
    tabs = [nc.declare_dram_parameter(f"tab{b}", [bpad, P], dt.bfloat16,
                                      isOutput=False) for b in range(nb)]
    idxs_d = nc.declare_dram_parameter("idxs", [P, nch * 8], dt.int16, isOutput=False)
    slots_d = nc.declare_dram_parameter("slots", [P, nch], dt.bfloat16, isOutput=False)
    vals_d = nc.declare_dram_parameter("vals", [P, nch], dt.bfloat16, isOutput=False)
    iota_d = nc.declare_dram_parameter("iota", [P, P], dt.bfloat16, isOutput=False)
    wt_d = nc.declare_dram_parameter("wt", [P, P], dt.bfloat16, isOutput=False)
    if has_bias:
        bias_d = nc.declare_dram_parameter("biasb", [P, P], dt.float32, isOutput=False)
    out_d = nc.declare_dram_parameter("out", [wpc * P, P], dt.float32, isOutput=True)

    with tile.TileContext(nc) as tc:
        with (
            tc.tile_pool(name="const", bufs=1) as constp,
            tc.tile_pool(name="edges", bufs=1) as edgep,
            tc.tile_pool(name="gat", bufs=4) as gatp,
            tc.tile_pool(name="cmp", bufs=2) as cmpp,
            tc.tile_pool(name="smat", bufs=2) as smatp,
            tc.tile_pool(name="o1", bufs=3) as o1p,
            tc.tile_pool(name="of", bufs=4) as ofp,
            tc.tile_pool(name="ps1", bufs=3, space="PSUM") as ps1p,
            tc.tile_pool(name="ps2", bufs=2, space="PSUM") as ps2p,
        ):
            iota_sb = constp.tile([P, P], dt.bfloat16, tag="iota")
            nc.sync.dma_start(out=iota_sb[:], in_=iota_d[:])
            wt_sb = constp.tile([P, P], dt.bfloat16, tag="wt")
            nc.sync.dma_start(out=wt_sb[:], in_=wt_d[:])
            if has_bias:
                bias_sb = constp.tile([P, P], dt.float32, tag="bias")
                nc.sync.dma_start(out=bias_sb[:], in_=bias_d[:])
            idxs_sb = edgep.tile([P, nch * 8], dt.int16, tag="idxs")
            nc.sync.dma_start(out=idxs_sb[:], in_=idxs_d[:])
            slots_sb = edgep.tile([P, nch], dt.bfloat16, tag="slots")
            nc.sync.dma_start(out=slots_sb[:], in_=slots_d[:])
            vals_sb = edgep.tile([P, nch], dt.bfloat16, tag="vals")
            nc.sync.dma_start(out=vals_sb[:], in_=vals_d[:])

            gc = wpg * C  # chunks per group
            for _rep in range(reps):
              for g in range(n_groups):
                gt = gatp.tile([P, gc * P], dt.bfloat16, tag="g")
                for b in range(nb):
                    cb = cbs[b]
                    if cb == 0:
                        continue
                    run0 = wpg * offs[b]          # chunk offset within group
                    ni = wpg * cb * P             # idxs in this call
                    nc.gpsimd.dma_gather(
                        out_ap=gt[:, run0 * P:(run0 + wpg * cb) * P].rearrange(
                            "p (k s) -> p k s", s=P),
                        in_ap=tabs[b][:],
                        idxs_ap=idxs_sb[:, (g * gc + run0) * 8:(g * gc + run0) * 8 + ni // 16],
                        num_idxs=ni,
                        num_idxs_reg=ni,
                        elem_size=P,
                        single_packet=(ni <= 1024),
                        queue_num=(g * nb + b) % nq,
                    )
                if os.environ.get("GCN_STAGE") == "gather":
                    for wi in range(wpg):
                        w = g * wpg + wi
                        of0 = ofp.tile([P, P], dt.float32, tag="of")
                        nc.vector.tensor_copy(out=of0[:], in_=gt[:, wi * P:(wi + 1) * P])
                        nc.sync.dma_start(out=out_d[w * P:(w + 1) * P, :], in_=of0[:])
                    continue
                c0 = g * gc
                cmp_t = cmpp.tile([P, gc * P], dt.bfloat16, tag="cmp")
                s_t = smatp.tile([P, gc * P], dt.bfloat16, tag="s")
                nc.vector.tensor_tensor(
                    out=cmp_t[:].rearrange("p (g s) -> p g s", s=P),
                    in0=slots_sb[:, c0:c0 + gc, None].to_broadcast([P, gc, P]),
                    in1=iota_sb[:, None, :].to_broadcast([P, gc, P]),
                    op=mybir.AluOpType.is_equal,
                )
                nc.vector.tensor_tensor(
                    out=s_t[:].rearrange("p (g s) -> p g s", s=P),
                    in0=cmp_t[:].rearrange("p (g s) -> p g s", s=P),
                    in1=vals_sb[:, c0:c0 + gc, None].to_broadcast([P, gc, P]),
                    op=mybir.AluOpType.mult,
                )
                for wi in range(wpg):
                    w = g * wpg + wi
                    # this window's chunk columns within the group tile
                    cols = []
                    for b in range(nb):
                        cb = cbs[b]
                        base = wpg * offs[b] + wi * cb
                        cols.extend(range(base, base + cb))
                    ps1 = ps1p.tile([P, P], dt.float32, tag="ps1")
                    for ci, c in enumerate(cols):
                        off = c * P
                        nc.tensor.matmul(
                            out=ps1[:],
                            lhsT=gt[:, off:off + P],
                            rhs=s_t[:, off:off + P],
                            start=(ci == 0),
                            stop=(ci == len(cols) - 1),
                        )
                    o1 = o1p.tile([P, P], dt.bfloat16, tag="o1")
                    nc.scalar.copy(out=o1[:], in_=ps1[:])
                    ps2 = ps2p.tile([P, P], dt.float32, tag="ps2")
                    nc.tensor.matmul(out=ps2[:], lhsT=o1[:], rhs=wt_sb[:],
                                     start=True, stop=True)
                    if has_bias:
                        tb = ofp.tile([P, P], dt.float32, tag="tb")
                        nc.vector.tensor_tensor(out=tb[:], in0=ps2[:], in1=bias_sb[:],
                                                op=mybir.AluOpType.add)
                        pre = tb
                    else:
                        pre = ps2
                    tm = ofp.tile([P, P], dt.float32, tag="tm")
                    nc.vector.tensor_scalar_mul(tm[:], pre[:], float(alpha))
                    of = ofp.tile([P, P], dt.float32, tag="of")
                    nc.vector.tensor_tensor(out=of[:], in0=pre[:], in1=tm[:],
                                            op=mybir.AluOpType.max)
                    nc.sync.dma_start(out=out_d[w * P:(w + 1) * P, :], in_=of[:])
    nc.compile()
    return nc


def _prep(seq2d, edge_val, edge_src, edge_dst):
    """Host-side: balance nodes into windows, bucket edges into chunk slots.

    Returns (per_core_arrays, node_row, wpc, nb, bpad, cbs)."""
    n = seq2d.shape[0]
    wpc = -(-n // (P * N_CORES))       # windows per core
    wpc = -(-wpc // 4) * 4             # round up so wpg=4 divides it
    tw = wpc * N_CORES                 # total windows
    nb = -(-n // MAX_BUCKET)           # gather-table buckets
    bpad = -(-n // nb)

    dst = edge_dst.astype(np.int64)
    src = edge_src.astype(np.int64)
    deg = np.bincount(dst, minlength=n)

    # serpentine assignment of degree-sorted nodes -> (window, slot)
    order = np.argsort(-deg, kind="stable")
    idx = np.arange(n)
    rnd = idx // tw
    pos = idx % tw
    wins = np.where(rnd % 2 == 0, pos, tw - 1 - pos)
    node_w = np.empty(n, np.int64)
    node_s = np.empty(n, np.int64)
    node_w[order] = wins
    node_s[order] = rnd
    node_row = node_w * P + node_s  # row in concatenated all-core output

    ew = node_w[dst]                # window of each edge
    ecore = ew // wpc
    ewl = ew % wpc
    eslot = node_s[dst]
    ebuck = src // bpad
    eloc = src % bpad

    wpg = 4 if wpc % 4 == 0 else (2 if wpc % 2 == 0 else 1)

    # per-(core, window, bucket) counts -> global per-bucket chunk maxima
    seg_of_edge = (ecore * wpc + ewl) * nb + ebuck
    cnt = np.bincount(seg_of_edge, minlength=tw * nb).reshape(tw, nb)
    cbs = tuple(int(x) for x in np.maximum(-(-cnt.max(axis=0) // P), 0))
    C = sum(cbs)
    offs = np.concatenate([[0], np.cumsum(cbs)]).astype(int)
    nch = wpc * C

    per_core = []
    for c in range(N_CORES):
        m = ecore == c
        key = ewl[m] * nb + ebuck[m]
        o = np.argsort(key, kind="stable")
        key = key[o]
        wl = ewl[m][o]
        bk = ebuck[m][o]
        sl = eslot[m][o]
        lo = eloc[m][o]
        vl = edge_val[m][o]
        scnt = np.bincount(key, minlength=wpc * nb)
        sstart = np.concatenate([[0], np.cumsum(scnt)[:-1]])
        pos_in_seg = np.arange(len(wl)) - sstart[key]
        g = wl // wpg
        wi = wl % wpg
        chunk = g * (wpg * C) + wpg * offs[bk] + wi * np.asarray(cbs)[bk] \
            + pos_in_seg // P
        lane = pos_in_seg % P
        assert (pos_in_seg < np.asarray(cbs)[bk] * P).all()

        flat = chunk * P + lane
        slot_a = np.zeros(nch * P, np.float32)
        val_a = np.zeros(nch * P, np.float32)
        slot_a[flat] = sl
        val_a[flat] = vl

        # idx array: call (g, b) is a contiguous run of wpg*C_b chunks;
        # within the call, element i sits at [i % 16, call_col0 + i // 16]
        run_start_chunk = g * (wpg * C) + wpg * offs[bk]
        i_in_call = (chunk - run_start_chunk) * P + lane
        col = run_start_chunk * 8 + i_in_call // 16
        row = i_in_call % 16
        idx_a = np.zeros((16, nch * 8), np.int16)
        idx_a[row, col] = lo
        idx_full = np.tile(idx_a, (8, 1))

        per_core.append((
            idx_full,
            np.ascontiguousarray(slot_a.reshape(nch, P).T).astype(_BF16_NP),
            np.ascontiguousarray(val_a.reshape(nch, P).T).astype(_BF16_NP),
        ))
    return per_core, node_row, wpc, nb, bpad, cbs


def kernel(seq, W, bias, prelu_a, edge_val, edge_src, edge_dst):
    global LAST_RESULTS
    seq = np.asarray(seq)
    W = np.asarray(W, dtype=np.float32)
    bias = np.asarray(bias, dtype=np.float32)
    alpha = float(np.asarray(prelu_a).reshape(-1)[0])
    assert 0.0 <= alpha <= 1.0, "prelu slope must be in [0,1] for the max() trick"
    edge_val = np.asarray(edge_val, dtype=np.float32)

    seq2d = np.ascontiguousarray(seq.reshape(-1, P).astype(np.float32))
    n = seq2d.shape[0]

    per_core, node_row, wpc, nb, bpad, cbs = _prep(
        seq2d, edge_val, np.asarray(edge_src), np.asarray(edge_dst))
    has_bias = bool(np.any(bias != 0.0))
    wpg = 4 if wpc % 4 == 0 else (2 if wpc % 2 == 0 else 1)

    cfg = (bpad, nb, cbs, wpc, wpg, alpha, has_bias)
    if cfg not in _prog_cache:
        _prog_cache[cfg] = _build_program(*cfg)
    nc = _prog_cache[cfg]

    seq_tab = seq2d.astype(_BF16_NP)
    tabs = {}
    for b in range(nb):
        t = seq_tab[b * bpad:(b + 1) * bpad]
        if t.shape[0] < bpad:
            t = np.vstack([t, np.zeros((bpad - t.shape[0], P), _BF16_NP)])
        tabs[f"tab{b}"] = np.ascontiguousarray(t)
    iota = np.tile(np.arange(P, dtype=np.float32), (P, 1)).astype(_BF16_NP)
    wt = np.ascontiguousarray(W.T).astype(_BF16_NP)
    in_maps = []
    for c in range(N_CORES):
        idx_a, slot_a, val_a = per_core[c]
        m = dict(tabs)
        m.update({
            "idxs": idx_a,
            "slots": slot_a,
            "vals": val_a,
            "iota": iota,
            "wt": wt,
        })
        if has_bias:
            m["biasb"] = np.ascontiguousarray(
                np.tile(bias.astype(np.float32), (P, 1)))
        in_maps.append(m)

    res = run_bass_kernel_spmd(nc, in_maps, list(range(N_CORES)))
    LAST_RESULTS = res

    flat = np.concatenate([res.results[c]["out"] for c in range(N_CORES)], axis=0)
    out = flat[node_row].astype(np.float32)
    _LAST_RUN["nc"] = nc
    _LAST_RUN["in_maps"] = in_maps
    _LAST_RUN["cfg"] = cfg
    return out.reshape(seq.shape[0], n, P) if seq.ndim == 3 else out


_LAST_RUN: dict = {}


def _time_program(nc, in_maps, iters: int = 50) -> dict:
    """Execute a program with device-resident inputs; return per-call wall
    times (ns).  'pipelined' issues all calls async then blocks once."""
    import time

    import jax
    from jax.sharding import Mesh, PartitionSpec
    from jax.experimental.shard_map import shard_map
    from concourse import bass2jax, mybir as mb

    bass2jax.install_neuronx_cc_hook()

    partition_name = nc.partition_id_tensor.name if nc.partition_id_tensor else None
    in_names, out_names, out_avals, zero_outs = [], [], [], []
    for alloc in nc.m.functions[0].allocations:
        if not isinstance(alloc, mb.MemoryLocationSet):
            continue
        name = alloc.memorylocations[0].name
        if alloc.kind == "ExternalInput":
            if name != partition_name:
                in_names.append(name)
        elif alloc.kind == "ExternalOutput":
            out_names.append(name)
            shape = tuple(alloc.tensor_shape)
            dtype = mb.dt.np(alloc.dtype)
            out_avals.append(jax.core.ShapedArray(shape, dtype))
            zero_outs.append(np.zeros(shape, dtype))
    n_params = len(in_names)
    all_in = list(in_names) + list(out_names)

    def _body(*args):
        operands = list(args)
        if partition_name is not None:
            operands.append(bass2jax.partition_id_tensor())
        return tuple(bass2jax._bass_exec_p.bind(
            *operands,
            out_avals=tuple(out_avals),
            in_names=tuple(all_in + ([partition_name] if partition_name else [])),
            out_names=tuple(out_names),
            lowering_input_output_aliases=(),
            sim_require_finite=True,
            sim_require_nnan=True,
            nc=nc,
        ))

    devices = jax.devices()[:N_CORES]
    mesh = Mesh(np.asarray(devices), ("core",))
    nin = n_params + len(zero_outs)
    sharded = jax.jit(shard_map(
        _body, mesh=mesh,
        in_specs=(PartitionSpec("core"),) * nin,
        out_specs=(PartitionSpec("core"),) * len(out_names),
        check_rep=False), keep_unused=True)

    sh = jax.sharding.NamedSharding(mesh, PartitionSpec("core"))
    dev_in = [jax.device_put(
        np.concatenate([np.asarray(in_maps[c][nm]) for c in range(N_CORES)], axis=0), sh)
        for nm in in_names]
    dev_zero = [jax.device_put(
        np.zeros((N_CORES * z.shape[0], *z.shape[1:]), z.dtype), sh)
        for z in zero_outs]

    out = sharded(*dev_in, *dev_zero)
    jax.block_until_ready(out)

    t0 = time.perf_counter()
    outs = [sharded(*dev_in, *dev_zero) for _ in range(iters)]
    jax.block_until_ready(outs)
    t_pipe = (time.perf_counter() - t0) / iters

    return {"pipelined_ns": t_pipe * 1e9}


def bench(iters: int = 50, reps: int = 3) -> dict:
    """Slope-based HW timing of the last kernel() call: time the program and a
    variant with the whole pipeline repeated `reps` times inside one NEFF;
    (T_reps - T_1) / (reps - 1) cancels per-execute dispatch overhead."""
    nc = _LAST_RUN["nc"]
    in_maps = _LAST_RUN["in_maps"]
    cfg = _LAST_RUN["cfg"]
    t1 = _time_program(nc, in_maps, iters)["pipelined_ns"]
    key = cfg + (reps,)
    if key not in _prog_cache:
        _prog_cache[key] = _build_program(*cfg, reps=reps)
    ncr = _prog_cache[key]
    tr = _time_program(ncr, in_maps, iters)["pipelined_ns"]
    slope = (tr - t1) / (reps - 1)
    return {"pipelined_ns": t1, "reps_ns": tr, "slope_ns": slope}


# revision 31
# speedup vs baseline: 4.1169x; 2.7551x over previous
"""GCN message-passing (gather + segment_sum + linear + PReLU) on 8 Trainium2 cores.

Strategy: shard destination nodes across cores.  Since segment_sum commutes
with the linear layer, raw seq features are aggregated first and W applied
after aggregation:

    out = prelu(segsum(val * seq[src]) @ W.T + bias)

Destination nodes are relabeled into (8 cores) x (wpc windows) x (128 slots),
degree-balanced so every window has a bounded number of incident edges.
Edges are grouped into chunks of 128 (one edge per SBUF partition).  Source
rows are fetched with dma_gather (int16 indices -> the seq table is split
into <=32767-row buckets; a window's edges are grouped by bucket).  For each
chunk a selection matrix S^T[e, s] = val[e] * (slot[e] == s) is built on the
DVE and the PE accumulates G.T @ S^T into PSUM [feat, slot] over the window.
One more matmul with W^T gives [slot, out_ft]; PReLU on the way out.
"""

import os
import sys

import numpy as np

for _p in ("/opt/trn_rl_repo", "/root/.axon_site/_ro/trn_rl_repo"):
    if os.path.isdir(_p) and _p not in sys.path:
        sys.path.insert(0, _p)

from concourse import bacc, bass, mybir, tile  # noqa: E402
from concourse.bass_utils import run_bass_kernel_spmd  # noqa: E402

P = 128
N_CORES = 8
MAX_BUCKET = 25000  # rows per gather table (int16 index limit is 32767)
_BF16_NP = mybir.dt.np(mybir.dt.bfloat16)

_prog_cache: dict = {}
LAST_RESULTS = None  # BassKernelResults of the most recent kernel() call


def _build_program(bpad: int, nb: int, cbs: tuple, wpc: int, wpg: int,
                   alpha: float, has_bias: bool, reps: int = 1) -> "bacc.Bacc":
    """cbs: per-bucket chunk counts (C_b).  Chunk grid per group of wpg
    windows: [b0: w0 chunks.. w_{wpg-1} chunks][b1: ...] ...  One dma_gather
    call per (group, bucket) covers its contiguous wpg*C_b chunk run."""
    dt = mybir.dt
    C = sum(cbs)
    nch = wpc * C
    offs = np.concatenate([[0], np.cumsum(cbs)]).astype(int)
    n_groups = wpc // wpg

    nq = int(os.environ.get("GCN_NQUEUES", "4"))
    nc = bacc.Bacc(num_swdge_queues=nq)
    tabs = [nc.declare_dram_parameter(f"tab{b}", [bpad, P], dt.bfloat16,
                                      isOutput=False) for b in range(nb)]
    idxs_d = nc.declare_dram_parameter("idxs", [P, nch * 8], dt.int16, isOutput=False)
    slots_d = nc.declare_dram_parameter("slots", [P, nch], dt.bfloat16, isOutput=False)
    vals_d = nc.declare_dram_parameter("vals", [P, nch], dt.bfloat16, isOutput=False)
    iota_d = nc.declare_dram_parameter("iota", [P, P], dt.bfloat16, isOutput=False)
    wt_d = nc.declare_dram_parameter("wt", [P, P], dt.bfloat16, isOutput=False)
    if has_bias:
        bias_d = nc.declare_dram_parameter("biasb", [P, P], dt.float32, isOutput=False)
    out_d = nc.declare_dram_parameter("out", [wpc * P, P], dt.float32, isOutput=True)

    with tile.TileContext(nc) as tc:
        with (
            tc.tile_pool(name="const", bufs=1) as constp,
            tc.tile_pool(name="edges", bufs=1) as edgep,
            tc.tile_pool(name="gat", bufs=4) as gatp,
            tc.tile_pool(name="cmp", bufs=2) as cmpp,
            tc.tile_pool(name="smat", bufs=2) as smatp,
            tc.tile_pool(name="o1", bufs=3) as o1p,
            tc.tile_pool(name="of", bufs=4) as ofp,
            tc.tile_pool(name="ps1", bufs=3, space="PSUM") as ps1p,
            tc.tile_pool(name="ps2", bufs=2, space="PSUM") as ps2p,
        ):
            iota_sb = constp.tile([P, P], dt.bfloat16, tag="iota")
            nc.sync.dma_start(out=iota_sb[:], in_=iota_d[:])
            wt_sb = constp.tile([P, P], dt.bfloat16, tag="wt")
            nc.sync.dma_start(out=wt_sb[:], in_=wt_d[:])
            if has_bias:
                bias_sb = constp.tile([P, P], dt.float32, tag="bias")
                nc.sync.dma_start(out=bias_sb[:], in_=bias_d[:])
            idxs_sb = edgep.tile([P, nch * 8], dt.int16, tag="idxs")
            nc.sync.dma_start(out=idxs_sb[:], in_=idxs_d[:])
            slots_sb = edgep.tile([P, nch], dt.bfloat16, tag="slots")
            nc.sync.dma_start(out=slots_sb[:], in_=slots_d[:])
            vals_sb = edgep.tile([P, nch], dt.bfloat16, tag="vals")
            nc.sync.dma_start(out=vals_sb[:], in_=vals_d[:])

            gc = wpg * C  # chunks per group
            for _rep in range(reps):
              for g in range(n_groups):
                gt = gatp.tile([P, gc * P], dt.bfloat16, tag="g")
                for b in range(nb):
                    cb = cbs[b]
                    if cb == 0:
                        continue
                    run0 = wpg * offs[b]          # chunk offset within group
                    ni = wpg * cb * P             # idxs in this call
                    nc.gpsimd.dma_gather(
                        out_ap=gt[:, run0 * P:(run0 + wpg * cb) * P].rearrange(
                            "p (k s) -> p k s", s=P),
                        in_ap=tabs[b][:],
                        idxs_ap=idxs_sb[:, (g * gc + run0) * 8:(g * gc + run0) * 8 + ni // 16],
                        num_idxs=ni,
                        num_idxs_reg=ni,
                        elem_size=P,
                        single_packet=(ni <= 1024),
                        queue_num=(g * nb + b) % nq,
                    )
                if os.environ.get("GCN_STAGE") == "gather":
                    for wi in range(wpg):
                        w = g * wpg + wi
                        of0 = ofp.tile([P, P], dt.float32, tag="of")
                        nc.vector.tensor_copy(out=of0[:], in_=gt[:, wi * P:(wi + 1) * P])
                        nc.sync.dma_start(out=out_d[w * P:(w + 1) * P, :], in_=of0[:])
                    continue
                c0 = g * gc
                cmp_t = cmpp.tile([P, gc * P], dt.bfloat16, tag="cmp")
                s_t = smatp.tile([P, gc * P], dt.bfloat16, tag="s")
                nc.vector.tensor_tensor(
                    out=cmp_t[:].rearrange("p (g s) -> p g s", s=P),
                    in0=slots_sb[:, c0:c0 + gc, None].to_broadcast([P, gc, P]),
                    in1=iota_sb[:, None, :].to_broadcast([P, gc, P]),
                    op=mybir.AluOpType.is_equal,
                )
                nc.vector.tensor_tensor(
                    out=s_t[:].rearrange("p (g s) -> p g s", s=P),
                    in0=cmp_t[:].rearrange("p (g s) -> p g s", s=P),
                    in1=vals_sb[:, c0:c0 + gc, None].to_broadcast([P, gc, P]),
                    op=mybir.AluOpType.mult,
                )
                for wi in range(wpg):
                    w = g * wpg + wi
                    # this window's chunk columns within the group tile
                    cols = []
                    for b in range(nb):
                        cb = cbs[b]
                        base = wpg * offs[b] + wi * cb
                        cols.extend(range(base, base + cb))
                    ps1 = ps1p.tile([P, P], dt.float32, tag="ps1")
                    for ci, c in enumerate(cols):
                        off = c * P
                        nc.tensor.matmul(
                            out=ps1[:],
                            lhsT=gt[:, off:off + P],
                            rhs=s_t[:, off:off + P],
                            start=(ci == 0),
                            stop=(ci == len(cols) - 1),
                        )
                    o1 = o1p.tile([P, P], dt.bfloat16, tag="o1")
                    nc.scalar.copy(out=o1[:], in_=ps1[:])
                    ps2 = ps2p.tile([P, P], dt.float32, tag="ps2")
                    nc.tensor.matmul(out=ps2[:], lhsT=o1[:], rhs=wt_sb[:],
                                     start=True, stop=True)
                    if has_bias:
                        tb = ofp.tile([P, P], dt.float32, tag="tb")
                        nc.vector.tensor_tensor(out=tb[:], in0=ps2[:], in1=bias_sb[:],
                                                op=mybir.AluOpType.add)
                        pre = tb
                    else:
                        pre = ps2
                    tm = ofp.tile([P, P], dt.float32, tag="tm")
                    nc.vector.tensor_scalar_mul(tm[:], pre[:], float(alpha))
                    of = ofp.tile([P, P], dt.float32, tag="of")
                    nc.vector.tensor_tensor(out=of[:], in0=pre[:], in1=tm[:],
                                            op=mybir.AluOpType.max)
                    nc.sync.dma_start(out=out_d[w * P:(w + 1) * P, :], in_=of[:])
    nc.compile()
    return nc


def _prep(seq2d, edge_val, edge_src, edge_dst):
    """Host-side: balance nodes into windows, bucket edges into chunk slots.

    Returns (per_core_arrays, node_row, wpc, nb, bpad, cbs)."""
    n = seq2d.shape[0]
    wpc = -(-n // (P * N_CORES))       # windows per core
    wpc = -(-wpc // 4) * 4             # round up so wpg=4 divides it
    tw = wpc * N_CORES                 # total windows

    dst = edge_dst.astype(np.int64)
    src = edge_src.astype(np.int64)
    deg = np.bincount(dst, minlength=n)

    # serpentine assignment of degree-sorted nodes -> (window, slot)
    order = np.argsort(-deg, kind="stable")
    idx = np.arange(n)
    rnd = idx // tw
    pos = idx % tw
    wins = np.where(rnd % 2 == 0, pos, tw - 1 - pos)
    node_w = np.empty(n, np.int64)
    node_s = np.empty(n, np.int64)
    node_w[order] = wins
    node_s[order] = rnd
    node_row = node_w * P + node_s  # row in concatenated all-core output

    ew = node_w[dst]                # window of each edge
    ecore = ew // wpc
    ewl = ew % wpc
    eslot = node_s[dst]

    wpg = 4 if wpc % 4 == 0 else (2 if wpc % 2 == 0 else 1)

    # choose the bucket width minimizing total chunks C (per-bucket maxima
    # over all (core, window) cells set the compile-time chunk grid)
    best = None
    for cand in (MAX_BUCKET, 32767, 30000, 28000):
        nb_c = -(-n // cand)
        bp_c = n if nb_c == 1 else cand
        seg = (ecore * wpc + ewl) * nb_c + src // bp_c
        cnt_c = np.bincount(seg, minlength=tw * nb_c).reshape(tw, nb_c)
        cbs_c = tuple(int(x) for x in np.maximum(-(-cnt_c.max(axis=0) // P), 0))
        if best is None or sum(cbs_c) < best[0]:
            best = (sum(cbs_c), bp_c, nb_c, cbs_c)
    C, bpad, nb, cbs = best
    ebuck = src // bpad
    eloc = src % bpad
    offs = np.concatenate([[0], np.cumsum(cbs)]).astype(int)
    nch = wpc * C

    per_core = []
    for c in range(N_CORES):
        m = ecore == c
        key = ewl[m] * nb + ebuck[m]
        o = np.argsort(key, kind="stable")
        key = key[o]
        wl = ewl[m][o]
        bk = ebuck[m][o]
        sl = eslot[m][o]
        lo = eloc[m][o]
        vl = edge_val[m][o]
        scnt = np.bincount(key, minlength=wpc * nb)
        sstart = np.concatenate([[0], np.cumsum(scnt)[:-1]])
        pos_in_seg = np.arange(len(wl)) - sstart[key]
        g = wl // wpg
        wi = wl % wpg
        chunk = g * (wpg * C) + wpg * offs[bk] + wi * np.asarray(cbs)[bk] \
            + pos_in_seg // P
        lane = pos_in_seg % P
        assert (pos_in_seg < np.asarray(cbs)[bk] * P).all()

        flat = chunk * P + lane
        slot_a = np.zeros(nch * P, np.float32)
        val_a = np.zeros(nch * P, np.float32)
        slot_a[flat] = sl
        val_a[flat] = vl

        # idx array: call (g, b) is a contiguous run of wpg*C_b chunks;
        # within the call, element i sits at [i % 16, call_col0 + i // 16]
        run_start_chunk = g * (wpg * C) + wpg * offs[bk]
        i_in_call = (chunk - run_start_chunk) * P + lane
        col = run_start_chunk * 8 + i_in_call // 16
        row = i_in_call % 16
        idx_a = np.zeros((16, nch * 8), np.int16)
        idx_a[row, col] = lo
        idx_full = np.tile(idx_a, (8, 1))

        per_core.append((
            idx_full,
            np.ascontiguousarray(slot_a.reshape(nch, P).T).astype(_BF16_NP),
            np.ascontiguousarray(val_a.reshape(nch, P).T).astype(_BF16_NP),
        ))
    return per_core, node_row, wpc, nb, bpad, cbs


def kernel(seq, W, bias, prelu_a, edge_val, edge_src, edge_dst):
    global LAST_RESULTS
    seq = np.asarray(seq)
    W = np.asarray(W, dtype=np.float32)
    bias = np.asarray(bias, dtype=np.float32)
    alpha = float(np.asarray(prelu_a).reshape(-1)[0])
    assert 0.0 <= alpha <= 1.0, "prelu slope must be in [0,1] for the max() trick"
    edge_val = np.asarray(edge_val, dtype=np.float32)

    seq2d = np.ascontiguousarray(seq.reshape(-1, P).astype(np.float32))
    n = seq2d.shape[0]

    per_core, node_row, wpc, nb, bpad, cbs = _prep(
        seq2d, edge_val, np.asarray(edge_src), np.asarray(edge_dst))
    has_bias = bool(np.any(bias != 0.0))
    wpg = 4 if wpc % 4 == 0 else (2 if wpc % 2 == 0 else 1)

    cfg = (bpad, nb, cbs, wpc, wpg, alpha, has_bias)
    if cfg not in _prog_cache:
        _prog_cache[cfg] = _build_program(*cfg)
    nc = _prog_cache[cfg]

    seq_tab = seq2d.astype(_BF16_NP)
    tabs = {}
    for b in range(nb):
        t = seq_tab[b * bpad:(b + 1) * bpad]
        if t.shape[0] < bpad:
            t = np.vstack([t, np.zeros((bpad - t.shape[0], P), _BF16_NP)])
        tabs[f"tab{b}"] = np.ascontiguousarray(t)
    iota = np.tile(np.arange(P, dtype=np.float32), (P, 1)).astype(_BF16_NP)
    wt = np.ascontiguousarray(W.T).astype(_BF16_NP)
    in_maps = []
    for c in range(N_CORES):
        idx_a, slot_a, val_a = per_core[c]
        m = dict(tabs)
        m.update({
            "idxs": idx_a,
            "slots": slot_a,
            "vals": val_a,
            "iota": iota,
            "wt": wt,
        })
        if has_bias:
            m["biasb"] = np.ascontiguousarray(
                np.tile(bias.astype(np.float32), (P, 1)))
        in_maps.append(m)

    res = run_bass_kernel_spmd(nc, in_maps, list(range(N_CORES)))
    LAST_RESULTS = res

    flat = np.concatenate([res.results[c]["out"] for c in range(N_CORES)], axis=0)
    out = flat[node_row].astype(np.float32)
    _LAST_RUN["nc"] = nc
    _LAST_RUN["in_maps"] = in_maps
    _LAST_RUN["cfg"] = cfg
    return out.reshape(seq.shape[0], n, P) if seq.ndim == 3 else out


_LAST_RUN: dict = {}


def _time_program(nc, in_maps, iters: int = 50) -> dict:
    """Execute a program with device-resident inputs; return per-call wall
    times (ns).  'pipelined' issues all calls async then blocks once."""
    import time

    import jax
    from jax.sharding import Mesh, PartitionSpec
    from jax.experimental.shard_map import shard_map
    from concourse import bass2jax, mybir as mb

    bass2jax.install_neuronx_cc_hook()

    partition_name = nc.partition_id_tensor.name if nc.partition_id_tensor else None
    in_names, out_names, out_avals, zero_outs = [], [], [], []
    for alloc in nc.m.functions[0].allocations:
        if not isinstance(alloc, mb.MemoryLocationSet):
            continue
        name = alloc.memorylocations[0].name
        if alloc.kind == "ExternalInput":
            if name != partition_name:
                in_names.append(name)
        elif alloc.kind == "ExternalOutput":
            out_names.append(name)
            shape = tuple(alloc.tensor_shape)
            dtype = mb.dt.np(alloc.dtype)
            out_avals.append(jax.core.ShapedArray(shape, dtype))
            zero_outs.append(np.zeros(shape, dtype))
    n_params = len(in_names)
    all_in = list(in_names) + list(out_names)

    def _body(*args):
        operands = list(args)
        if partition_name is not None:
            operands.append(bass2jax.partition_id_tensor())
        return tuple(bass2jax._bass_exec_p.bind(
            *operands,
            out_avals=tuple(out_avals),
            in_names=tuple(all_in + ([partition_name] if partition_name else [])),
            out_names=tuple(out_names),
            lowering_input_output_aliases=(),
            sim_require_finite=True,
            sim_require_nnan=True,
            nc=nc,
        ))

    devices = jax.devices()[:N_CORES]
    mesh = Mesh(np.asarray(devices), ("core",))
    nin = n_params + len(zero_outs)
    sharded = jax.jit(shard_map(
        _body, mesh=mesh,
        in_specs=(PartitionSpec("core"),) * nin,
        out_specs=(PartitionSpec("core"),) * len(out_names),
        check_rep=False), keep_unused=True)

    sh = jax.sharding.NamedSharding(mesh, PartitionSpec("core"))
    dev_in = [jax.device_put(
        np.concatenate([np.asarray(in_maps[c][nm]) for c in range(N_CORES)], axis=0), sh)
        for nm in in_names]
    dev_zero = [jax.device_put(
        np.zeros((N_CORES * z.shape[0], *z.shape[1:]), z.dtype), sh)
        for z in zero_outs]

    out = sharded(*dev_in, *dev_zero)
    jax.block_until_ready(out)

    t0 = time.perf_counter()
    outs = [sharded(*dev_in, *dev_zero) for _ in range(iters)]
    jax.block_until_ready(outs)
    t_pipe = (time.perf_counter() - t0) / iters

    return {"pipelined_ns": t_pipe * 1e9}


def bench(iters: int = 50, reps: int = 3) -> dict:
    """Slope-based HW timing of the last kernel() call: time the program and a
    variant with the whole pipeline repeated `reps` times inside one NEFF;
    (T_reps - T_1) / (reps - 1) cancels per-execute dispatch overhead."""
    nc = _LAST_RUN["nc"]
    in_maps = _LAST_RUN["in_maps"]
    cfg = _LAST_RUN["cfg"]
    t1 = _time_program(nc, in_maps, iters)["pipelined_ns"]
    key = cfg + (reps,)
    if key not in _prog_cache:
        _prog_cache[key] = _build_program(*cfg, reps=reps)
    ncr = _prog_cache[key]
    tr = _time_program(ncr, in_maps, iters)["pipelined_ns"]
    slope = (tr - t1) / (reps - 1)
    return {"pipelined_ns": t1, "reps_ns": tr, "slope_ns": slope}
